# revision 13
# baseline (speedup 1.0000x reference)
"""Trainium2 Bass kernel for nn_CrossAttention (B=8, T=S=C=1024, H=16).

Sharding: pure data-parallel over batch B — batch element b runs on core b.
No collectives needed.

Per-core pipeline (all attention math in "transposed world" so no per-block
transposes are needed in the attention inner loop):
  1. PE-transpose x -> xT [C, T] and feature -> fT [C, S] (128x128 blocks).
  2. qT[c,t]  = Wq^T . xT   (weights stationary, fp32r)
     kT[c,s]  = Wk^T . fT
     v[s,c]   = fT^T . Wv   (natural layout, +ones column per head for the
                             fused softmax denominator)
  3. attT[s,t] = kT-block (stationary) x qT (moving); causal slicing skips
     dead blocks.  exp on ScalarE (no max subtraction needed: logits are
     bounded ~ +-4 for this problem's scale).  Triangular mask on the
     diagonal blocks.
  4. avT[d,t] accumulated over s-blocks: lhsT = [v_h | ones] so row 64 of the
     PSUM result is the softmax denominator for free.
  5. Per-head normalization: DVE reciprocal of the denominator row, PE
     broadcast (K=1 matmul) across the 64 d-partitions, DVE multiply.
  6. out[t,c] = avn^T (stationary) . Wp^T  + bp, natural layout -> DMA out.

All matmuls run in float32r (1 cycle/row at N>=256, ~1e-4 relative error).
"""

import os

import numpy as np

import concourse.mybir as mybir
import concourse.tile as tile
from concourse import bacc
from concourse.bass_utils import run_bass_kernel_spmd

F32 = mybir.dt.float32
F32R = mybir.dt.float32r
AF = mybir.ActivationFunctionType
P = 128
T = 1024          # query positions
S = 1024          # key positions
C = 1024          # channels
H = 16            # heads
D = C // H        # 64 head dim
KC = C // P       # 8 contraction chunks
TB = T // P       # 8 t-blocks
NF = 512          # matmul free-dim chunk
NTC = T // NF     # 2 t 512-chunks

_CACHE = {}

# Debug staging: 1=qT, 2=kT, 3=v, 4=partial-head avn, 5=all avn, 6=full kernel
STAGE = int(os.environ.get("KSTAGE", "6"))
NHEADS = int(os.environ.get("KHEADS", str(H)))


def _build(stage=None, nheads=None):
    stage = STAGE if stage is None else stage
    nheads = NHEADS if nheads is None else nheads
    nc = bacc.Bacc(None, debug=False)

    def din(name, shape, dt=F32R):
        return nc.declare_dram_parameter(name, list(shape), dt, isOutput=False).ap()

    aps = {
        "xb": din("xb", [T, C]),
        "fb": din("fb", [S, C]),
        "wq": din("wq", [C, C]),            # (Wc*scale).T  [k, c]
        "wk": din("wk", [C, C]),            # Wf[:C].T      [k, c]
        "wv": din("wv", [C, C]),            # Wf[C:].T      [k, c]
        "wp": din("wp", [C, C]),            # Wp.T          [k, c]
        "bq2": din("bq2", [P, KC], F32),    # bc*scale as [128, 8]
        "bk2": din("bk2", [P, KC], F32),    # bf[:C] as [128, 8]
        "bv_b": din("bv_b", [P, C], F32),   # bf[C:] broadcast over partitions
        "bp_b": din("bp_b", [P, C], F32),   # bp broadcast over partitions
        "tri": din("tri", [P, P]),          # tri[s,t] = 1 if t >= s else 0
        "ident": din("ident", [P, P]),      # identity for PE transposes
        "ones64": din("ones64", [1, 64]),   # K=1 broadcast matmul lhsT
        "onescol": din("onescol", [P, H]),  # ones columns for v_aug
        "ob": nc.declare_dram_parameter("ob", [T, C], F32, isOutput=True).ap(),
    }

    with tile.TileContext(nc) as tc:
        with (
            tc.tile_pool(name="consts", bufs=1) as consts,
            tc.tile_pool(name="scratch", bufs=4) as scratch,     # x_raw/f_raw/exp-big
            tc.tile_pool(name="trans", bufs=8) as trans,         # xT -> fT
            tc.tile_pool(name="qtp", bufs=8) as qtp,             # qT -> avn
            tc.tile_pool(name="ktp", bufs=8) as ktp,
            tc.tile_pool(name="vp", bufs=8) as vpool,
            tc.tile_pool(name="wpool", bufs=8) as wpool,
            tc.tile_pool(name="exps", bufs=4) as exps,           # exp small [128,512]
            tc.tile_pool(name="avsb", bufs=1) as avsb,
            tc.tile_pool(name="recipp", bufs=2) as recipp,
            tc.tile_pool(name="outp", bufs=2) as outp,
            tc.tile_pool(name="attps", bufs=2, space="PSUM") as attps,   # + transposes
            tc.tile_pool(name="projps", bufs=2, space="PSUM") as projps,
            tc.tile_pool(name="avps", bufs=2, space="PSUM") as avps,
            tc.tile_pool(name="bcps", bufs=2, space="PSUM") as bcps,
        ):
            pools = {
                "consts": consts, "scratch": scratch, "trans": trans,
                "qtp": qtp, "ktp": ktp, "vp": vpool, "wpool": wpool,
                "exps": exps, "avsb": avsb, "recipp": recipp, "outp": outp,
                "attps": attps, "projps": projps, "avps": avps, "bcps": bcps,
            }
            _emit(nc, stage, nheads, aps, pools)
    nc.compile()
    return nc


def _emit(nc, stage, nheads, aps, pools):
    consts = pools["consts"]; scratch = pools["scratch"]; trans = pools["trans"]
    qtp = pools["qtp"]; ktp = pools["ktp"]; vpool = pools["vp"]
    wpool = pools["wpool"]; exps = pools["exps"]; avsb = pools["avsb"]
    recipp = pools["recipp"]; outp = pools["outp"]
    attps = pools["attps"]; projps = pools["projps"]
    avps = pools["avps"]; bcps = pools["bcps"]
    xb = aps["xb"]; fb = aps["fb"]; ob = aps["ob"]

    def dump_rows(src_ap, row0):
        o_ = outp.tile([P, T], F32, name="dbg", tag="dbg", bufs=1)
        nc.vector.tensor_copy(o_[:], src_ap)
        nc.sync.dma_start(out=ob[row0:row0 + P, :], in_=o_[:])

    # ---- constants ----
    tri_sb = consts.tile([P, P], F32R, name="tri_sb")
    nc.sync.dma_start(out=tri_sb[:], in_=aps["tri"][:])
    id_sb = consts.tile([P, P], F32R, name="id_sb")
    nc.sync.dma_start(out=id_sb[:], in_=aps["ident"][:])
    ones_sb = consts.tile([1, 64], F32R, name="ones_sb")
    nc.sync.dma_start(out=ones_sb[:], in_=aps["ones64"][:])
    bq_sb = consts.tile([P, KC], F32, name="bq_sb")
    nc.sync.dma_start(out=bq_sb[:], in_=aps["bq2"][:])
    bk_sb = consts.tile([P, KC], F32, name="bk_sb")
    nc.sync.dma_start(out=bk_sb[:], in_=aps["bk2"][:])
    bv_sb = consts.tile([P, C], F32, name="bv_sb", tag="bias_b")
    nc.sync.dma_start(out=bv_sb[:], in_=aps["bv_b"][:])

    def transpose_into(raw_tiles, dst_tiles):
        # raw [t,k] blocks -> dst [k,t]; dst[kc][:, tb*128:...]
        for tb in range(TB):
            for kc in range(KC):
                tp = attps.tile([P, NF], F32R, name="tp", tag="attps")
                nc.tensor.transpose(
                    tp[:, :P], raw_tiles[tb][:, kc * P:(kc + 1) * P], id_sb[:]
                )
                nc.vector.tensor_copy(
                    dst_tiles[kc][:, tb * P:(tb + 1) * P], tp[:, :P]
                )

    # ---- load + transpose x ----
    x_raw = []
    for i in range(TB):
        t_ = scratch.tile([P, C], F32R, name=f"xr{i}", tag="big")
        nc.sync.dma_start(out=t_[:], in_=xb[i * P:(i + 1) * P, :])
        x_raw.append(t_)
    xT = [trans.tile([P, T], F32R, name=f"xT{k}", tag="tr") for k in range(KC)]
    transpose_into(x_raw, xT)

    # ---- q projection: qT[cc] [128, T] ----
    wq_sb = []
    for k in range(KC):
        w_ = wpool.tile([P, C], F32R, name=f"wq{k}", tag="w")
        nc.sync.dma_start(out=w_[:], in_=aps["wq"][k * P:(k + 1) * P, :])
        wq_sb.append(w_)
    qT = []
    for cc in range(KC):
        qt = qtp.tile([P, T], F32R, name=f"qT{cc}", tag="qT")
        qT.append(qt)
        for tc in range(NTC):
            ps = projps.tile([P, NF], F32, name="qps", tag="proj")
            for kc in range(KC):
                nc.tensor.matmul(
                    ps[:],
                    wq_sb[kc][:, cc * P:(cc + 1) * P],
                    xT[kc][:, tc * NF:(tc + 1) * NF],
                    start=(kc == 0), stop=(kc == KC - 1),
                )
            nc.vector.tensor_scalar_add(
                qt[:, tc * NF:(tc + 1) * NF], ps[:], bq_sb[:, cc:cc + 1]
            )
    if stage == 1:
        for cc in range(KC):
            dump_rows(qT[cc][:].bitcast(F32), cc * P)
        return

    # ---- load + transpose feature (reuses scratch + trans slots) ----
    f_raw = []
    for i in range(TB):
        t_ = scratch.tile([P, C], F32R, name=f"fr{i}", tag="big")
        nc.sync.dma_start(out=t_[:], in_=fb[i * P:(i + 1) * P, :])
        f_raw.append(t_)
    fT = [trans.tile([P, S], F32R, name=f"fT{k}", tag="tr") for k in range(KC)]
    transpose_into(f_raw, fT)

    # ---- k projection ----
    wk_sb = []
    for k in range(KC):
        w_ = wpool.tile([P, C], F32R, name=f"wk{k}", tag="w")
        nc.sync.dma_start(out=w_[:], in_=aps["wk"][k * P:(k + 1) * P, :])
        wk_sb.append(w_)
    kT = []
    for cc in range(KC):
        kt = ktp.tile([P, S], F32R, name=f"kT{cc}", tag="kT")
        kT.append(kt)
        for tc in range(NTC):
            ps = projps.tile([P, NF], F32, name="kps", tag="proj")
            for kc in range(KC):
                nc.tensor.matmul(
                    ps[:],
                    wk_sb[kc][:, cc * P:(cc + 1) * P],
                    fT[kc][:, tc * NF:(tc + 1) * NF],
                    start=(kc == 0), stop=(kc == KC - 1),
                )
            nc.vector.tensor_scalar_add(
                kt[:, tc * NF:(tc + 1) * NF], ps[:], bk_sb[:, cc:cc + 1]
            )
    if stage == 2:
        for cc in range(KC):
            dump_rows(kT[cc][:].bitcast(F32), cc * P)
        return

    # ---- v projection: v_sb[sb] [128, H, D+1], col D = ones ----
    wv_sb = []
    for k in range(KC):
        w_ = wpool.tile([P, C], F32R, name=f"wv{k}", tag="w")
        nc.sync.dma_start(out=w_[:], in_=aps["wv"][k * P:(k + 1) * P, :])
        wv_sb.append(w_)
    v_sb = []
    for sb in range(TB):
        vt = vpool.tile([P, H, D + 1], F32R, name=f"v{sb}", tag="v")
        v_sb.append(vt)
        nc.sync.dma_start(out=vt[:, :, D:D + 1], in_=aps["onescol"][:, :, None])
        for c2 in range(NTC):
            ps = projps.tile([P, NF], F32, name="vps", tag="proj")
            for kc in range(KC):
                nc.tensor.matmul(
                    ps[:],
                    fT[kc][:, sb * P:(sb + 1) * P],
                    wv_sb[kc][:, c2 * NF:(c2 + 1) * NF],
                    start=(kc == 0), stop=(kc == KC - 1),
                )
            nc.vector.tensor_tensor(
                vt[:, c2 * 8:(c2 + 1) * 8, 0:D],
                ps[:].rearrange("p (a b) -> p a b", b=D),
                bv_sb[:, c2 * NF:(c2 + 1) * NF].rearrange("p (a b) -> p a b", b=D),
                mybir.AluOpType.add,
            )
    if stage == 3:
        for sb in range(TB):
            o_ = outp.tile([P, T], F32, name="dbg", tag="dbg", bufs=1)
            nc.vector.tensor_copy(
                o_[:].rearrange("p (a b) -> p a b", b=D),
                v_sb[sb][:, :, 0:D].bitcast(F32),
            )
            nc.sync.dma_start(out=ob[sb * P:(sb + 1) * P, :], in_=o_[:])
        return

    # ---- attention per head ----
    avn = [qtp.tile([P, T], F32R, name=f"avn{cc}", tag="qT") for cc in range(KC)]
    for h in range(nheads):
        cc_h = h // 2
        off = (h % 2) * D
        # attT + exp per s-block
        exp_t = []
        for sb in range(TB):
            cd = sb // 4                 # 512-chunk containing the diagonal
            width = T - cd * NF          # live width (512-aligned)
            loff = sb * P - cd * NF      # live offset inside the tile
            tag = "big" if width > NF else "exps"
            pool = scratch if width > NF else exps
            et = pool.tile([P, width], F32R, name=f"e{h}_{sb}", tag=tag)
            exp_t.append(et)
            for j in range(width // NF):
                aps_ = attps.tile([P, NF], F32, name="aps", tag="attps")
                nc.tensor.matmul(
                    aps_[:],
                    kT[cc_h][off:off + D, sb * P:(sb + 1) * P],
                    qT[cc_h][off:off + D, cd * NF + j * NF: cd * NF + (j + 1) * NF],
                    start=True, stop=True,
                )
                lo = max(loff - j * NF, 0)
                nc.scalar.activation(
                    et[:, j * NF + lo:(j + 1) * NF],
                    aps_[:, lo:],
                    AF.Exp,
                )
            # mask the diagonal 128-block
            nc.vector.tensor_tensor(
                et[:, loff:loff + P], et[:, loff:loff + P], tri_sb[:],
                mybir.AluOpType.mult,
            )
        # avT accumulation per t 512-chunk
        av_ps = []
        for tc in range(NTC):
            ap_ = avps.tile([D + 1, NF], F32, name="avp", tag="av")
            av_ps.append(ap_)
            first = True
            for sb in range(4 * tc + 4):
                cd = sb // 4
                if cd > tc:
                    continue
                loff = sb * P - cd * NF
                rs = loff if cd == tc else 0
                src_off = (tc - cd) * NF
                last = (sb == 4 * tc + 3)
                nc.tensor.matmul(
                    ap_[:, rs:NF],
                    v_sb[sb][:, h, :],
                    exp_t[sb][:, src_off + rs:src_off + NF],
                    start=first, stop=last,
                )
                first = False
        # broadcast recip across 64 partitions (K=1 matmul) + normalize
        tmp = None
        if off != 0:
            tmp = avsb.tile([D, T], F32R, name="avtmp", tag="avtmp")
        for tc in range(NTC):
            rc = recipp.tile([1, NF], F32R, name="rc", tag="rc")
            with nc.allow_low_precision("fp32r softmax denominators"):
                nc.vector.reciprocal(rc[:], av_ps[tc][D:D + 1, :])
            bc = bcps.tile([D, NF], F32, name="bc", tag="bc")
            nc.tensor.matmul(
                bc[:], ones_sb[:], rc[:], start=True, stop=True,
            )
            av_f = avsb.tile([D, NF], F32, name="avf", tag="avf", bufs=2)
            nc.scalar.activation(av_f[:], av_ps[tc][0:D, :], AF.Copy)
            dst = avn[cc_h][0:D, tc * NF:(tc + 1) * NF] if off == 0 \
                else tmp[:, tc * NF:(tc + 1) * NF]
            nc.vector.tensor_tensor(dst, av_f[:], bc[:], mybir.AluOpType.mult)
        if off != 0:
            nc.sync.dma_start(out=avn[cc_h][D:2 * D, :], in_=tmp[:])
    if stage in (4, 5):
        for cc in range(KC if stage == 5 else (nheads + 1) // 2):
            dump_rows(avn[cc][:].bitcast(F32), cc * P)
        return

    # ---- output projection ----
    bp_sb = consts.tile([P, C], F32, name="bp_sb", tag="bias_b")
    nc.sync.dma_start(out=bp_sb[:], in_=aps["bp_b"][:])
    wp_sb = []
    for k in range(KC):
        w_ = wpool.tile([P, C], F32R, name=f"wp{k}", tag="w")
        nc.sync.dma_start(out=w_[:], in_=aps["wp"][k * P:(k + 1) * P, :])
        wp_sb.append(w_)
    for tb in range(TB):
        for co in range(NTC):
            ps = projps.tile([P, NF], F32, name="ops", tag="proj")
            for kc in range(KC):
                nc.tensor.matmul(
                    ps[:],
                    avn[kc][:, tb * P:(tb + 1) * P],
                    wp_sb[kc][:, co * NF:(co + 1) * NF],
                    start=(kc == 0), stop=(kc == KC - 1),
                )
            ot = outp.tile([P, NF], F32, name="ot", tag="ot")
            nc.vector.tensor_tensor(
                ot[:], ps[:], bp_sb[:, co * NF:(co + 1) * NF],
                mybir.AluOpType.add,
            )
            nc.sync.dma_start(
                out=ob[tb * P:(tb + 1) * P, co * NF:(co + 1) * NF], in_=ot[:]
            )


def _prep_consts(Wc, bc, Wf, bf, Wp, bp):
    scale = 1.0 / np.sqrt(np.float32(D))
    f32 = np.float32
    consts = {
        "wq": np.ascontiguousarray((Wc * scale).T, dtype=f32),
        "wk": np.ascontiguousarray(Wf[:C].T, dtype=f32),
        "wv": np.ascontiguousarray(Wf[C:].T, dtype=f32),
        "wp": np.ascontiguousarray(Wp.T, dtype=f32),
        "bq2": np.ascontiguousarray((bc * scale).reshape(KC, P).T, dtype=f32),
        "bk2": np.ascontiguousarray(bf[:C].reshape(KC, P).T, dtype=f32),
        "bv_b": np.ascontiguousarray(np.broadcast_to(bf[C:], (P, C)), dtype=f32),
        "bp_b": np.ascontiguousarray(np.broadcast_to(bp, (P, C)), dtype=f32),
        "tri": np.ascontiguousarray(np.triu(np.ones((P, P), dtype=f32))),
        "ident": np.eye(P, dtype=f32),
        "ones64": np.ones((1, 64), dtype=f32),
        "onescol": np.ones((P, H), dtype=f32),
    }
    return consts


def kernel(x, feature, Wc, bc, Wf, bf, Wp, bp, _trace=False, _out=None):
    x = np.asarray(x, dtype=np.float32)
    feature = np.asarray(feature, dtype=np.float32)
    if "nc" not in _CACHE:
        _CACHE["nc"] = _build()
    nc = _CACHE["nc"]
    consts = _prep_consts(
        np.asarray(Wc, np.float32), np.asarray(bc, np.float32),
        np.asarray(Wf, np.float32), np.asarray(bf, np.float32),
        np.asarray(Wp, np.float32), np.asarray(bp, np.float32),
    )
    B = x.shape[0]
    in_maps = [
        {"xb": np.ascontiguousarray(x[b]), "fb": np.ascontiguousarray(feature[b]), **consts}
        for b in range(B)
    ]
    res = run_bass_kernel_spmd(nc, in_maps, list(range(B)), trace=_trace)
    if _out is not None:
        _out.append(res)
    out = np.stack([res.results[b]["ob"] for b in range(B)], axis=0)
    return out


# revision 14
# speedup vs baseline: 1.1066x; 1.1066x over previous
"""Trainium2 Bass kernel for nn_CrossAttention (B=8, T=S=C=1024, H=16).

Sharding: pure data-parallel over batch B — batch element b runs on core b.
No collectives needed.

Per-core pipeline (all attention math in "transposed world" so no per-block
transposes are needed in the attention inner loop):
  1. PE-transpose x -> xT [C, T] and feature -> fT [C, S] (128x128 blocks).
  2. qT[c,t]  = Wq^T . xT   (weights stationary, fp32r)
     kT[c,s]  = Wk^T . fT
     v[s,c]   = fT^T . Wv   (natural layout, +ones column per head for the
                             fused softmax denominator)
  3. attT[s,t] = kT-block (stationary) x qT (moving); causal slicing skips
     dead blocks.  exp on ScalarE (no max subtraction needed: logits are
     bounded ~ +-4 for this problem's scale).  Triangular mask on the
     diagonal blocks.
  4. avT[d,t] accumulated over s-blocks: lhsT = [v_h | ones] so row 64 of the
     PSUM result is the softmax denominator for free.
  5. Per-head normalization: DVE reciprocal of the denominator row, PE
     broadcast (K=1 matmul) across the 64 d-partitions, DVE multiply.
  6. out[t,c] = avn^T (stationary) . Wp^T  + bp, natural layout -> DMA out.

All matmuls run in float32r (1 cycle/row at N>=256, ~1e-4 relative error).
"""

import os

import numpy as np

import concourse.mybir as mybir
import concourse.tile as tile
from concourse import bacc
from concourse.bass_utils import run_bass_kernel_spmd

F32 = mybir.dt.float32
F32R = mybir.dt.float32r
AF = mybir.ActivationFunctionType
P = 128
T = 1024          # query positions
S = 1024          # key positions
C = 1024          # channels
H = 16            # heads
D = C // H        # 64 head dim
KC = C // P       # 8 contraction chunks
TB = T // P       # 8 t-blocks
NF = 512          # matmul free-dim chunk
NTC = T // NF     # 2 t 512-chunks

_CACHE = {}

# Debug staging: 1=qT, 2=kT, 3=v, 4=partial-head avn, 5=all avn, 6=full kernel
STAGE = int(os.environ.get("KSTAGE", "6"))
NHEADS = int(os.environ.get("KHEADS", str(H)))
KDTYPE = os.environ.get("KDTYPE", "f32r")
BF16 = mybir.dt.bfloat16
DT = BF16 if KDTYPE == "bf16" else F32R


def _build(stage=None, nheads=None):
    stage = STAGE if stage is None else stage
    nheads = NHEADS if nheads is None else nheads
    nc = bacc.Bacc(None, debug=False)

    def din(name, shape, dt=DT):
        return nc.declare_dram_parameter(name, list(shape), dt, isOutput=False).ap()

    aps = {
        "xb": din("xb", [T, C]),
        "fb": din("fb", [S, C]),
        "wq": din("wq", [C, C]),            # (Wc*scale).T  [k, c]
        "wk": din("wk", [C, C]),            # Wf[:C].T      [k, c]
        "wv": din("wv", [C, C]),            # Wf[C:].T      [k, c]
        "wp": din("wp", [C, C]),            # Wp.T          [k, c]
        "bq2": din("bq2", [P, KC], F32),    # bc*scale as [128, 8]
        "bk2": din("bk2", [P, KC], F32),    # bf[:C] as [128, 8]
        "bv_b": din("bv_b", [P, C], F32),   # bf[C:] broadcast over partitions
        "bp_b": din("bp_b", [P, C], F32),   # bp broadcast over partitions
        "tri": din("tri", [P, P]),          # tri[s,t] = 1 if t >= s else 0
        "ident": din("ident", [P, P]),      # identity for PE transposes
        "ones64": din("ones64", [1, 64], F32R),   # K=1 broadcast matmul lhsT
        "onescol": din("onescol", [P, H]),  # ones columns for v_aug
        "ob": nc.declare_dram_parameter("ob", [T, C], F32, isOutput=True).ap(),
    }

    with tile.TileContext(nc) as tc:
        with (
            tc.tile_pool(name="consts", bufs=1) as consts,
            tc.tile_pool(name="scratch", bufs=4) as scratch,     # x_raw/f_raw/exp-big
            tc.tile_pool(name="trans", bufs=8) as trans,         # xT -> fT
            tc.tile_pool(name="qtp", bufs=8) as qtp,             # qT -> avn
            tc.tile_pool(name="ktp", bufs=8) as ktp,
            tc.tile_pool(name="vp", bufs=8) as vpool,
            tc.tile_pool(name="wpool", bufs=8) as wpool,
            tc.tile_pool(name="exps", bufs=4) as exps,           # exp small [128,512]
            tc.tile_pool(name="avsb", bufs=1) as avsb,
            tc.tile_pool(name="recipp", bufs=2) as recipp,
            tc.tile_pool(name="outp", bufs=2) as outp,
            tc.tile_pool(name="attps", bufs=2, space="PSUM") as attps,   # + transposes
            tc.tile_pool(name="projps", bufs=2, space="PSUM") as projps,
            tc.tile_pool(name="avps", bufs=2, space="PSUM") as avps,
            tc.tile_pool(name="bcps", bufs=2, space="PSUM") as bcps,
        ):
            pools = {
                "consts": consts, "scratch": scratch, "trans": trans,
                "qtp": qtp, "ktp": ktp, "vp": vpool, "wpool": wpool,
                "exps": exps, "avsb": avsb, "recipp": recipp, "outp": outp,
                "attps": attps, "projps": projps, "avps": avps, "bcps": bcps,
            }
            _emit(nc, stage, nheads, aps, pools)
    nc.compile()
    return nc


def _emit(nc, stage, nheads, aps, pools):
    consts = pools["consts"]; scratch = pools["scratch"]; trans = pools["trans"]
    qtp = pools["qtp"]; ktp = pools["ktp"]; vpool = pools["vp"]
    wpool = pools["wpool"]; exps = pools["exps"]; avsb = pools["avsb"]
    recipp = pools["recipp"]; outp = pools["outp"]
    attps = pools["attps"]; projps = pools["projps"]
    avps = pools["avps"]; bcps = pools["bcps"]
    xb = aps["xb"]; fb = aps["fb"]; ob = aps["ob"]

    def dump_rows(src_ap, row0):
        o_ = outp.tile([P, T], F32, name="dbg", tag="dbg", bufs=1)
        nc.vector.tensor_copy(o_[:], src_ap)
        nc.sync.dma_start(out=ob[row0:row0 + P, :], in_=o_[:])

    # ---- constants ----
    tri_sb = consts.tile([P, P], DT, name="tri_sb")
    nc.sync.dma_start(out=tri_sb[:], in_=aps["tri"][:])
    id_sb = consts.tile([P, P], DT, name="id_sb")
    nc.sync.dma_start(out=id_sb[:], in_=aps["ident"][:])
    ones_sb = consts.tile([1, 64], F32R, name="ones_sb")
    nc.sync.dma_start(out=ones_sb[:], in_=aps["ones64"][:])
    bq_sb = consts.tile([P, KC], F32, name="bq_sb")
    nc.sync.dma_start(out=bq_sb[:], in_=aps["bq2"][:])
    bk_sb = consts.tile([P, KC], F32, name="bk_sb")
    nc.sync.dma_start(out=bk_sb[:], in_=aps["bk2"][:])
    bv_sb = consts.tile([P, C], F32, name="bv_sb", tag="bias_b")
    nc.sync.dma_start(out=bv_sb[:], in_=aps["bv_b"][:])

    def transpose_into(raw_tiles, dst_tiles):
        # raw [t,k] blocks -> dst [k,t]; dst[kc][:, tb*128:...]
        for tb in range(TB):
            for kc in range(KC):
                tp = attps.tile([P, NF], F32R, name="tp", tag="attps")
                nc.tensor.transpose(
                    tp[:, :P], raw_tiles[tb][:, kc * P:(kc + 1) * P], id_sb[:]
                )
                nc.vector.tensor_copy(
                    dst_tiles[kc][:, tb * P:(tb + 1) * P], tp[:, :P]
                )

    # ---- load + transpose x ----
    xT = [trans.tile([P, T], DT, name=f"xT{k}", tag="tr") for k in range(KC)]
    if DT == BF16:
        for k in range(KC):
            nc.sync.dma_start_transpose(xT[k][:], xb[:, k * P:(k + 1) * P])
    else:
        x_raw = []
        for i in range(TB):
            t_ = scratch.tile([P, C], F32R, name=f"xr{i}", tag="big")
            nc.sync.dma_start(out=t_[:], in_=xb[i * P:(i + 1) * P, :])
            x_raw.append(t_)
        transpose_into(x_raw, xT)

    # ---- q projection: qT[cc] [128, T] ----
    wq_sb = []
    for k in range(KC):
        w_ = wpool.tile([P, C], DT, name=f"wq{k}", tag="w")
        nc.sync.dma_start(out=w_[:], in_=aps["wq"][k * P:(k + 1) * P, :])
        wq_sb.append(w_)
    qT = []
    for cc in range(KC):
        qt = qtp.tile([P, T], DT, name=f"qT{cc}", tag="qT")
        qT.append(qt)
        for tc in range(NTC):
            ps = projps.tile([P, NF], F32, name="qps", tag="proj")
            for kc in range(KC):
                nc.tensor.matmul(
                    ps[:],
                    wq_sb[kc][:, cc * P:(cc + 1) * P],
                    xT[kc][:, tc * NF:(tc + 1) * NF],
                    start=(kc == 0), stop=(kc == KC - 1),
                )
            nc.vector.tensor_scalar_add(
                qt[:, tc * NF:(tc + 1) * NF], ps[:], bq_sb[:, cc:cc + 1]
            )
    if stage == 1:
        for cc in range(KC):
            dump_rows(qT[cc][:].bitcast(F32), cc * P)
        return

    # ---- load + transpose feature (reuses scratch + trans slots) ----
    fT = [trans.tile([P, S], DT, name=f"fT{k}", tag="tr") for k in range(KC)]
    if DT == BF16:
        for k in range(KC):
            nc.sync.dma_start_transpose(fT[k][:], fb[:, k * P:(k + 1) * P])
    else:
        f_raw = []
        for i in range(TB):
            t_ = scratch.tile([P, C], F32R, name=f"fr{i}", tag="big")
            nc.sync.dma_start(out=t_[:], in_=fb[i * P:(i + 1) * P, :])
            f_raw.append(t_)
        transpose_into(f_raw, fT)

    # ---- k projection ----
    wk_sb = []
    for k in range(KC):
        w_ = wpool.tile([P, C], DT, name=f"wk{k}", tag="w")
        nc.sync.dma_start(out=w_[:], in_=aps["wk"][k * P:(k + 1) * P, :])
        wk_sb.append(w_)
    kT = []
    for cc in range(KC):
        kt = ktp.tile([P, S], DT, name=f"kT{cc}", tag="kT")
        kT.append(kt)
        for tc in range(NTC):
            ps = projps.tile([P, NF], F32, name="kps", tag="proj")
            for kc in range(KC):
                nc.tensor.matmul(
                    ps[:],
                    wk_sb[kc][:, cc * P:(cc + 1) * P],
                    fT[kc][:, tc * NF:(tc + 1) * NF],
                    start=(kc == 0), stop=(kc == KC - 1),
                )
            nc.vector.tensor_scalar_add(
                kt[:, tc * NF:(tc + 1) * NF], ps[:], bk_sb[:, cc:cc + 1]
            )
    if stage == 2:
        for cc in range(KC):
            dump_rows(kT[cc][:].bitcast(F32), cc * P)
        return

    # ---- v projection: v_sb[sb] [128, H, D+1], col D = ones ----
    wv_sb = []
    for k in range(KC):
        w_ = wpool.tile([P, C], DT, name=f"wv{k}", tag="w")
        nc.sync.dma_start(out=w_[:], in_=aps["wv"][k * P:(k + 1) * P, :])
        wv_sb.append(w_)
    v_sb = []
    for sb in range(TB):
        vt = vpool.tile([P, H, D + 1], DT, name=f"v{sb}", tag="v")
        v_sb.append(vt)
        nc.sync.dma_start(out=vt[:, :, D:D + 1], in_=aps["onescol"][:, :, None])
        for c2 in range(NTC):
            ps = projps.tile([P, NF], F32, name="vps", tag="proj")
            for kc in range(KC):
                nc.tensor.matmul(
                    ps[:],
                    fT[kc][:, sb * P:(sb + 1) * P],
                    wv_sb[kc][:, c2 * NF:(c2 + 1) * NF],
                    start=(kc == 0), stop=(kc == KC - 1),
                )
            nc.vector.tensor_tensor(
                vt[:, c2 * 8:(c2 + 1) * 8, 0:D],
                ps[:].rearrange("p (a b) -> p a b", b=D),
                bv_sb[:, c2 * NF:(c2 + 1) * NF].rearrange("p (a b) -> p a b", b=D),
                mybir.AluOpType.add,
            )
    if stage == 3:
        for sb in range(TB):
            o_ = outp.tile([P, T], F32, name="dbg", tag="dbg", bufs=1)
            nc.vector.tensor_copy(
                o_[:].rearrange("p (a b) -> p a b", b=D),
                v_sb[sb][:, :, 0:D].bitcast(F32),
            )
            nc.sync.dma_start(out=ob[sb * P:(sb + 1) * P, :], in_=o_[:])
        return

    # ---- attention per head ----
    avn = [qtp.tile([P, T], DT, name=f"avn{cc}", tag="qT") for cc in range(KC)]
    for h in range(nheads):
        cc_h = h // 2
        off = (h % 2) * D
        # attT + exp per s-block
        exp_t = []
        for sb in range(TB):
            cd = sb // 4                 # 512-chunk containing the diagonal
            width = T - cd * NF          # live width (512-aligned)
            loff = sb * P - cd * NF      # live offset inside the tile
            tag = "big" if width > NF else "exps"
            pool = scratch if width > NF else exps
            et = pool.tile([P, width], DT, name=f"e{h}_{sb}", tag=tag)
            exp_t.append(et)
            for j in range(width // NF):
                aps_ = attps.tile([P, NF], F32, name="aps", tag="attps")
                nc.tensor.matmul(
                    aps_[:],
                    kT[cc_h][off:off + D, sb * P:(sb + 1) * P],
                    qT[cc_h][off:off + D, cd * NF + j * NF: cd * NF + (j + 1) * NF],
                    start=True, stop=True,
                )
                lo = max(loff - j * NF, 0)
                nc.scalar.activation(
                    et[:, j * NF + lo:(j + 1) * NF],
                    aps_[:, lo:],
                    AF.Exp,
                )
            # mask the diagonal 128-block
            nc.vector.tensor_tensor(
                et[:, loff:loff + P], et[:, loff:loff + P], tri_sb[:],
                mybir.AluOpType.mult,
            )
        # avT accumulation per t 512-chunk
        av_ps = []
        for tc in range(NTC):
            ap_ = avps.tile([D + 1, NF], F32, name="avp", tag="av")
            av_ps.append(ap_)
            first = True
            for sb in range(4 * tc + 4):
                cd = sb // 4
                if cd > tc:
                    continue
                loff = sb * P - cd * NF
                rs = loff if cd == tc else 0
                src_off = (tc - cd) * NF
                last = (sb == 4 * tc + 3)
                nc.tensor.matmul(
                    ap_[:, rs:NF],
                    v_sb[sb][:, h, :],
                    exp_t[sb][:, src_off + rs:src_off + NF],
                    start=first, stop=last,
                )
                first = False
        # broadcast recip across 64 partitions (K=1 matmul) + normalize
        tmp = None
        if off != 0:
            tmp = avsb.tile([D, T], DT, name="avtmp", tag="avtmp")
        for tc in range(NTC):
            rc = recipp.tile([1, NF], F32R, name="rc", tag="rc")
            with nc.allow_low_precision("fp32r softmax denominators"):
                nc.vector.reciprocal(rc[:], av_ps[tc][D:D + 1, :])
            bc = bcps.tile([D, NF], F32, name="bc", tag="bc")
            nc.tensor.matmul(
                bc[:], ones_sb[:], rc[:], start=True, stop=True,
            )
            av_f = avsb.tile([D, NF], F32, name="avf", tag="avf", bufs=2)
            nc.scalar.activation(av_f[:], av_ps[tc][0:D, :], AF.Copy)
            dst = avn[cc_h][0:D, tc * NF:(tc + 1) * NF] if off == 0 \
                else tmp[:, tc * NF:(tc + 1) * NF]
            nc.vector.tensor_tensor(dst, av_f[:], bc[:], mybir.AluOpType.mult)
        if off != 0:
            nc.sync.dma_start(out=avn[cc_h][D:2 * D, :], in_=tmp[:])
    if stage in (4, 5):
        for cc in range(KC if stage == 5 else (nheads + 1) // 2):
            dump_rows(avn[cc][:].bitcast(F32), cc * P)
        return

    # ---- output projection ----
    bp_sb = consts.tile([P, C], F32, name="bp_sb", tag="bias_b")
    nc.sync.dma_start(out=bp_sb[:], in_=aps["bp_b"][:])
    wp_sb = []
    for k in range(KC):
        w_ = wpool.tile([P, C], DT, name=f"wp{k}", tag="w")
        nc.sync.dma_start(out=w_[:], in_=aps["wp"][k * P:(k + 1) * P, :])
        wp_sb.append(w_)
    for tb in range(TB):
        for co in range(NTC):
            ps = projps.tile([P, NF], F32, name="ops", tag="proj")
            for kc in range(KC):
                nc.tensor.matmul(
                    ps[:],
                    avn[kc][:, tb * P:(tb + 1) * P],
                    wp_sb[kc][:, co * NF:(co + 1) * NF],
                    start=(kc == 0), stop=(kc == KC - 1),
                )
            ot = outp.tile([P, NF], F32, name="ot", tag="ot")
            nc.vector.tensor_tensor(
                ot[:], ps[:], bp_sb[:, co * NF:(co + 1) * NF],
                mybir.AluOpType.add,
            )
            nc.sync.dma_start(
                out=ob[tb * P:(tb + 1) * P, co * NF:(co + 1) * NF], in_=ot[:]
            )


def _np_dt():
    if DT == BF16:
        import ml_dtypes
        return ml_dtypes.bfloat16
    return np.float32


def _prep_consts(Wc, bc, Wf, bf, Wp, bp):
    scale = 1.0 / np.sqrt(np.float32(D))
    f32 = np.float32
    ndt = _np_dt()
    consts = {
        "wq": np.ascontiguousarray((Wc * scale).T).astype(ndt),
        "wk": np.ascontiguousarray(Wf[:C].T).astype(ndt),
        "wv": np.ascontiguousarray(Wf[C:].T).astype(ndt),
        "wp": np.ascontiguousarray(Wp.T).astype(ndt),
        "bq2": np.ascontiguousarray((bc * scale).reshape(KC, P).T, dtype=f32),
        "bk2": np.ascontiguousarray(bf[:C].reshape(KC, P).T, dtype=f32),
        "bv_b": np.ascontiguousarray(np.broadcast_to(bf[C:], (P, C)), dtype=f32),
        "bp_b": np.ascontiguousarray(np.broadcast_to(bp, (P, C)), dtype=f32),
        "tri": np.triu(np.ones((P, P), dtype=f32)).astype(ndt),
        "ident": np.eye(P, dtype=f32).astype(ndt),
        "ones64": np.ones((1, 64), dtype=f32),
        "onescol": np.ones((P, H), dtype=f32).astype(ndt),
    }
    return consts


def kernel(x, feature, Wc, bc, Wf, bf, Wp, bp, _trace=False, _out=None):
    x = np.asarray(x, dtype=np.float32)
    feature = np.asarray(feature, dtype=np.float32)
    if "nc" not in _CACHE:
        _CACHE["nc"] = _build()
    nc = _CACHE["nc"]
    consts = _prep_consts(
        np.asarray(Wc, np.float32), np.asarray(bc, np.float32),
        np.asarray(Wf, np.float32), np.asarray(bf, np.float32),
        np.asarray(Wp, np.float32), np.asarray(bp, np.float32),
    )
    B = x.shape[0]
    ndt = _np_dt()
    in_maps = [
        {"xb": np.ascontiguousarray(x[b]).astype(ndt),
         "fb": np.ascontiguousarray(feature[b]).astype(ndt), **consts}
        for b in range(B)
    ]
    res = run_bass_kernel_spmd(nc, in_maps, list(range(B)), trace=_trace)
    if _out is not None:
        _out.append(res)
    out = np.stack([res.results[b]["ob"] for b in range(B)], axis=0)
    return out


# revision 15
# speedup vs baseline: 1.1499x; 1.0391x over previous
"""Trainium2 Bass kernel for nn_CrossAttention (B=8, T=S=C=1024, H=16).

Sharding: pure data-parallel over batch B — batch element b runs on core b.
No collectives needed.

Per-core pipeline (all attention math in "transposed world" so no per-block
transposes are needed in the attention inner loop):
  1. PE-transpose x -> xT [C, T] and feature -> fT [C, S] (128x128 blocks).
  2. qT[c,t]  = Wq^T . xT   (weights stationary, fp32r)
     kT[c,s]  = Wk^T . fT
     v[s,c]   = fT^T . Wv   (natural layout, +ones column per head for the
                             fused softmax denominator)
  3. attT[s,t] = kT-block (stationary) x qT (moving); causal slicing skips
     dead blocks.  exp on ScalarE (no max subtraction needed: logits are
     bounded ~ +-4 for this problem's scale).  Triangular mask on the
     diagonal blocks.
  4. avT[d,t] accumulated over s-blocks: lhsT = [v_h | ones] so row 64 of the
     PSUM result is the softmax denominator for free.
  5. Per-head normalization: DVE reciprocal of the denominator row, PE
     broadcast (K=1 matmul) across the 64 d-partitions, DVE multiply.
  6. out[t,c] = avn^T (stationary) . Wp^T  + bp, natural layout -> DMA out.

All matmuls run in float32r (1 cycle/row at N>=256, ~1e-4 relative error).
"""

import os

import numpy as np

import concourse.mybir as mybir
import concourse.tile as tile
from concourse import bacc
from concourse.bass_utils import run_bass_kernel_spmd

F32 = mybir.dt.float32
F32R = mybir.dt.float32r
AF = mybir.ActivationFunctionType
P = 128
T = 1024          # query positions
S = 1024          # key positions
C = 1024          # channels
H = 16            # heads
D = C // H        # 64 head dim
KC = C // P       # 8 contraction chunks
TB = T // P       # 8 t-blocks
NF = 512          # matmul free-dim chunk
NTC = T // NF     # 2 t 512-chunks

_CACHE = {}

# Debug staging: 1=qT, 2=kT, 3=v, 4=partial-head avn, 5=all avn, 6=full kernel
STAGE = int(os.environ.get("KSTAGE", "6"))
NHEADS = int(os.environ.get("KHEADS", str(H)))
KDTYPE = os.environ.get("KDTYPE", "f32r")
BF16 = mybir.dt.bfloat16
DT = BF16 if KDTYPE == "bf16" else F32R


def _build(stage=None, nheads=None):
    stage = STAGE if stage is None else stage
    nheads = NHEADS if nheads is None else nheads
    nc = bacc.Bacc(None, debug=False)

    def din(name, shape, dt=DT):
        return nc.declare_dram_parameter(name, list(shape), dt, isOutput=False).ap()

    aps = {
        "xb": din("xb", [T, C]),
        "fb": din("fb", [S, C]),
        "wq": din("wq", [C, C]),            # (Wc*scale).T  [k, c]
        "wk": din("wk", [C, C]),            # Wf[:C].T      [k, c]
        "wv": din("wv", [C, C]),            # Wf[C:].T      [k, c]
        "wp": din("wp", [C, C]),            # Wp.T          [k, c]
        "bq2": din("bq2", [P, KC], F32),    # bc*scale as [128, 8]
        "bk2": din("bk2", [P, KC], F32),    # bf[:C] as [128, 8]
        "bv_b": din("bv_b", [P, C], F32),   # bf[C:] broadcast over partitions
        "bp_b": din("bp_b", [P, C], F32),   # bp broadcast over partitions
        "tri": din("tri", [P, P]),          # tri[s,t] = 1 if t >= s else 0
        "ident": din("ident", [P, P]),      # identity for PE transposes
        "ones64": din("ones64", [1, 64], F32R),   # K=1 broadcast matmul lhsT
        "onescol": din("onescol", [P, H]),  # ones columns for v_aug
        "ob": nc.declare_dram_parameter("ob", [T, C], F32, isOutput=True).ap(),
    }

    with tile.TileContext(nc) as tc:
        with (
            tc.tile_pool(name="consts", bufs=1) as consts,
            tc.tile_pool(name="scratch", bufs=8) as scratch,     # x_raw/f_raw/exp-big
            tc.tile_pool(name="trans", bufs=8) as trans,         # xT -> fT
            tc.tile_pool(name="qtp", bufs=8) as qtp,             # qT -> avn
            tc.tile_pool(name="ktp", bufs=8) as ktp,
            tc.tile_pool(name="vp", bufs=8) as vpool,
            tc.tile_pool(name="wpool", bufs=8) as wpool,
            tc.tile_pool(name="exps", bufs=8) as exps,           # exp small [128,512]
            tc.tile_pool(name="avsb", bufs=2) as avsb,
            tc.tile_pool(name="recipp", bufs=2) as recipp,
            tc.tile_pool(name="outp", bufs=4) as outp,
            tc.tile_pool(name="attps", bufs=2, space="PSUM") as attps,   # + transposes
            tc.tile_pool(name="projps", bufs=2, space="PSUM") as projps,
            tc.tile_pool(name="avps", bufs=3, space="PSUM") as avps,
            tc.tile_pool(name="bcps", bufs=1, space="PSUM") as bcps,
        ):
            pools = {
                "consts": consts, "scratch": scratch, "trans": trans,
                "qtp": qtp, "ktp": ktp, "vp": vpool, "wpool": wpool,
                "exps": exps, "avsb": avsb, "recipp": recipp, "outp": outp,
                "attps": attps, "projps": projps, "avps": avps, "bcps": bcps,
            }
            _emit(nc, stage, nheads, aps, pools)
    nc.compile()
    return nc


def _emit(nc, stage, nheads, aps, pools):
    consts = pools["consts"]; scratch = pools["scratch"]; trans = pools["trans"]
    qtp = pools["qtp"]; ktp = pools["ktp"]; vpool = pools["vp"]
    wpool = pools["wpool"]; exps = pools["exps"]; avsb = pools["avsb"]
    recipp = pools["recipp"]; outp = pools["outp"]
    attps = pools["attps"]; projps = pools["projps"]
    avps = pools["avps"]; bcps = pools["bcps"]
    xb = aps["xb"]; fb = aps["fb"]; ob = aps["ob"]

    def dump_rows(src_ap, row0):
        o_ = outp.tile([P, T], F32, name="dbg", tag="dbg", bufs=1)
        nc.vector.tensor_copy(o_[:], src_ap)
        nc.sync.dma_start(out=ob[row0:row0 + P, :], in_=o_[:])

    # ---- constants ----
    tri_sb = consts.tile([P, P], DT, name="tri_sb")
    nc.sync.dma_start(out=tri_sb[:], in_=aps["tri"][:])
    id_sb = consts.tile([P, P], DT, name="id_sb")
    nc.sync.dma_start(out=id_sb[:], in_=aps["ident"][:])
    ones_sb = consts.tile([1, 64], F32R, name="ones_sb")
    nc.sync.dma_start(out=ones_sb[:], in_=aps["ones64"][:])
    bq_sb = consts.tile([P, KC], F32, name="bq_sb")
    nc.sync.dma_start(out=bq_sb[:], in_=aps["bq2"][:])
    bk_sb = consts.tile([P, KC], F32, name="bk_sb")
    nc.sync.dma_start(out=bk_sb[:], in_=aps["bk2"][:])
    bv_sb = consts.tile([P, C], F32, name="bv_sb", tag="bias_b")
    nc.sync.dma_start(out=bv_sb[:], in_=aps["bv_b"][:])

    def transpose_into(raw_tiles, dst_tiles):
        # raw [t,k] blocks -> dst [k,t]; dst[kc][:, tb*128:...]
        for tb in range(TB):
            for kc in range(KC):
                tp = attps.tile([P, NF], F32R, name="tp", tag="attps")
                nc.tensor.transpose(
                    tp[:, :P], raw_tiles[tb][:, kc * P:(kc + 1) * P], id_sb[:]
                )
                nc.vector.tensor_copy(
                    dst_tiles[kc][:, tb * P:(tb + 1) * P], tp[:, :P]
                )

    # ---- load + transpose x ----
    xT = [trans.tile([P, T], DT, name=f"xT{k}", tag="tr") for k in range(KC)]
    if DT == BF16:
        for k in range(KC):
            nc.sync.dma_start_transpose(xT[k][:], xb[:, k * P:(k + 1) * P])
    else:
        x_raw = []
        for i in range(TB):
            t_ = scratch.tile([P, C], F32R, name=f"xr{i}", tag="big")
            nc.sync.dma_start(out=t_[:], in_=xb[i * P:(i + 1) * P, :])
            x_raw.append(t_)
        transpose_into(x_raw, xT)

    # ---- q projection: qT[cc] [128, T] ----
    wq_sb = []
    for k in range(KC):
        w_ = wpool.tile([P, C], DT, name=f"wq{k}", tag="w")
        nc.sync.dma_start(out=w_[:], in_=aps["wq"][k * P:(k + 1) * P, :])
        wq_sb.append(w_)
    qT = []
    for cc in range(KC):
        qt = qtp.tile([P, T], DT, name=f"qT{cc}", tag="qT")
        qT.append(qt)
        for tc in range(NTC):
            ps = projps.tile([P, NF], F32, name="qps", tag="proj")
            for kc in range(KC):
                nc.tensor.matmul(
                    ps[:],
                    wq_sb[kc][:, cc * P:(cc + 1) * P],
                    xT[kc][:, tc * NF:(tc + 1) * NF],
                    start=(kc == 0), stop=(kc == KC - 1),
                )
            nc.vector.tensor_scalar_add(
                qt[:, tc * NF:(tc + 1) * NF], ps[:], bq_sb[:, cc:cc + 1]
            )
    if stage == 1:
        for cc in range(KC):
            dump_rows(qT[cc][:].bitcast(F32), cc * P)
        return

    # ---- load + transpose feature (reuses scratch + trans slots) ----
    ftag = "trf" if DT == BF16 else "tr"
    fT = [trans.tile([P, S], DT, name=f"fT{k}", tag=ftag) for k in range(KC)]
    if DT == BF16:
        for k in range(KC):
            nc.sync.dma_start_transpose(fT[k][:], fb[:, k * P:(k + 1) * P])
    else:
        f_raw = []
        for i in range(TB):
            t_ = scratch.tile([P, C], F32R, name=f"fr{i}", tag="big")
            nc.sync.dma_start(out=t_[:], in_=fb[i * P:(i + 1) * P, :])
            f_raw.append(t_)
        transpose_into(f_raw, fT)

    # ---- k projection ----
    wk_sb = []
    for k in range(KC):
        w_ = wpool.tile([P, C], DT, name=f"wk{k}", tag="w")
        nc.sync.dma_start(out=w_[:], in_=aps["wk"][k * P:(k + 1) * P, :])
        wk_sb.append(w_)
    kT = []
    for cc in range(KC):
        kt = ktp.tile([P, S], DT, name=f"kT{cc}", tag="kT")
        kT.append(kt)
        for tc in range(NTC):
            ps = projps.tile([P, NF], F32, name="kps", tag="proj")
            for kc in range(KC):
                nc.tensor.matmul(
                    ps[:],
                    wk_sb[kc][:, cc * P:(cc + 1) * P],
                    fT[kc][:, tc * NF:(tc + 1) * NF],
                    start=(kc == 0), stop=(kc == KC - 1),
                )
            nc.vector.tensor_scalar_add(
                kt[:, tc * NF:(tc + 1) * NF], ps[:], bk_sb[:, cc:cc + 1]
            )
    if stage == 2:
        for cc in range(KC):
            dump_rows(kT[cc][:].bitcast(F32), cc * P)
        return

    # ---- v projection: v_sb[sb] [128, H, D+1], col D = ones ----
    wv_sb = []
    for k in range(KC):
        w_ = wpool.tile([P, C], DT, name=f"wv{k}", tag="w")
        nc.sync.dma_start(out=w_[:], in_=aps["wv"][k * P:(k + 1) * P, :])
        wv_sb.append(w_)
    v_sb = []
    for sb in range(TB):
        vt = vpool.tile([P, H, D + 1], DT, name=f"v{sb}", tag="v")
        v_sb.append(vt)
        nc.sync.dma_start(out=vt[:, :, D:D + 1], in_=aps["onescol"][:, :, None])
        for c2 in range(NTC):
            ps = projps.tile([P, NF], F32, name="vps", tag="proj")
            for kc in range(KC):
                nc.tensor.matmul(
                    ps[:],
                    fT[kc][:, sb * P:(sb + 1) * P],
                    wv_sb[kc][:, c2 * NF:(c2 + 1) * NF],
                    start=(kc == 0), stop=(kc == KC - 1),
                )
            nc.vector.tensor_tensor(
                vt[:, c2 * 8:(c2 + 1) * 8, 0:D],
                ps[:].rearrange("p (a b) -> p a b", b=D),
                bv_sb[:, c2 * NF:(c2 + 1) * NF].rearrange("p (a b) -> p a b", b=D),
                mybir.AluOpType.add,
            )
    if stage == 3:
        for sb in range(TB):
            o_ = outp.tile([P, T], F32, name="dbg", tag="dbg", bufs=1)
            nc.vector.tensor_copy(
                o_[:].rearrange("p (a b) -> p a b", b=D),
                v_sb[sb][:, :, 0:D].bitcast(F32),
            )
            nc.sync.dma_start(out=ob[sb * P:(sb + 1) * P, :], in_=o_[:])
        return

    # ---- attention per head ----
    avntag = "avn" if DT == BF16 else "qT"
    avn = [qtp.tile([P, T], DT, name=f"avn{cc}", tag=avntag) for cc in range(KC)]
    for h in range(nheads):
        cc_h = h // 2
        off = (h % 2) * D
        # attT + exp per s-block
        exp_t = []
        for sb in range(TB):
            cd = sb // 4                 # 512-chunk containing the diagonal
            width = T - cd * NF          # live width (512-aligned)
            loff = sb * P - cd * NF      # live offset inside the tile
            tag = "big" if width > NF else "exps"
            pool = scratch if width > NF else exps
            et = pool.tile([P, width], DT, name=f"e{h}_{sb}", tag=tag)
            exp_t.append(et)
            for j in range(width // NF):
                aps_ = attps.tile([P, NF], F32, name="aps", tag="attps")
                nc.tensor.matmul(
                    aps_[:],
                    kT[cc_h][off:off + D, sb * P:(sb + 1) * P],
                    qT[cc_h][off:off + D, cd * NF + j * NF: cd * NF + (j + 1) * NF],
                    start=True, stop=True,
                )
                lo = max(loff - j * NF, 0)
                nc.scalar.activation(
                    et[:, j * NF + lo:(j + 1) * NF],
                    aps_[:, lo:],
                    AF.Exp,
                )
            # mask the diagonal 128-block
            nc.vector.tensor_tensor(
                et[:, loff:loff + P], et[:, loff:loff + P], tri_sb[:],
                mybir.AluOpType.mult,
            )
        # avT accumulation per t 512-chunk
        av_ps = []
        for tc in range(NTC):
            ap_ = avps.tile([D + 1, NF], F32, name="avp", tag="av")
            av_ps.append(ap_)
            first = True
            for sb in range(4 * tc + 4):
                cd = sb // 4
                if cd > tc:
                    continue
                loff = sb * P - cd * NF
                rs = loff if cd == tc else 0
                src_off = (tc - cd) * NF
                last = (sb == 4 * tc + 3)
                nc.tensor.matmul(
                    ap_[:, rs:NF],
                    v_sb[sb][:, h, :],
                    exp_t[sb][:, src_off + rs:src_off + NF],
                    start=first, stop=last,
                )
                first = False
        # broadcast recip across 64 partitions (K=1 matmul) + normalize
        tmp = None
        if off != 0:
            tmp = avsb.tile([D, T], DT, name="avtmp", tag="avtmp")
        for tc in range(NTC):
            rc = recipp.tile([1, NF], F32R, name="rc", tag="rc")
            with nc.allow_low_precision("fp32r softmax denominators"):
                nc.vector.reciprocal(rc[:], av_ps[tc][D:D + 1, :])
            bc = bcps.tile([D, NF], F32, name="bc", tag="bc")
            nc.tensor.matmul(
                bc[:], ones_sb[:], rc[:], start=True, stop=True,
            )
            av_f = avsb.tile([D, NF], F32, name="avf", tag="avf", bufs=2)
            nc.scalar.activation(av_f[:], av_ps[tc][0:D, :], AF.Copy)
            dst = avn[cc_h][0:D, tc * NF:(tc + 1) * NF] if off == 0 \
                else tmp[:, tc * NF:(tc + 1) * NF]
            nc.vector.tensor_tensor(dst, av_f[:], bc[:], mybir.AluOpType.mult)
        if off != 0:
            nc.gpsimd.dma_start(out=avn[cc_h][D:2 * D, :], in_=tmp[:])
    if stage in (4, 5):
        for cc in range(KC if stage == 5 else (nheads + 1) // 2):
            dump_rows(avn[cc][:].bitcast(F32), cc * P)
        return

    # ---- output projection ----
    bp_sb = consts.tile([P, C], F32, name="bp_sb", tag="bias_b")
    nc.sync.dma_start(out=bp_sb[:], in_=aps["bp_b"][:])
    wp_sb = []
    for k in range(KC):
        w_ = wpool.tile([P, C], DT, name=f"wp{k}", tag="w")
        nc.sync.dma_start(out=w_[:], in_=aps["wp"][k * P:(k + 1) * P, :])
        wp_sb.append(w_)
    for tb in range(TB):
        for co in range(NTC):
            ps = projps.tile([P, NF], F32, name="ops", tag="proj")
            for kc in range(KC):
                nc.tensor.matmul(
                    ps[:],
                    avn[kc][:, tb * P:(tb + 1) * P],
                    wp_sb[kc][:, co * NF:(co + 1) * NF],
                    start=(kc == 0), stop=(kc == KC - 1),
                )
            ot = outp.tile([P, NF], F32, name="ot", tag="ot")
            nc.vector.tensor_tensor(
                ot[:], ps[:], bp_sb[:, co * NF:(co + 1) * NF],
                mybir.AluOpType.add,
            )
            nc.sync.dma_start(
                out=ob[tb * P:(tb + 1) * P, co * NF:(co + 1) * NF], in_=ot[:]
            )


def _np_dt():
    if DT == BF16:
        import ml_dtypes
        return ml_dtypes.bfloat16
    return np.float32


def _prep_consts(Wc, bc, Wf, bf, Wp, bp):
    scale = 1.0 / np.sqrt(np.float32(D))
    f32 = np.float32
    ndt = _np_dt()
    consts = {
        "wq": np.ascontiguousarray((Wc * scale).T).astype(ndt),
        "wk": np.ascontiguousarray(Wf[:C].T).astype(ndt),
        "wv": np.ascontiguousarray(Wf[C:].T).astype(ndt),
        "wp": np.ascontiguousarray(Wp.T).astype(ndt),
        "bq2": np.ascontiguousarray((bc * scale).reshape(KC, P).T, dtype=f32),
        "bk2": np.ascontiguousarray(bf[:C].reshape(KC, P).T, dtype=f32),
        "bv_b": np.ascontiguousarray(np.broadcast_to(bf[C:], (P, C)), dtype=f32),
        "bp_b": np.ascontiguousarray(np.broadcast_to(bp, (P, C)), dtype=f32),
        "tri": np.triu(np.ones((P, P), dtype=f32)).astype(ndt),
        "ident": np.eye(P, dtype=f32).astype(ndt),
        "ones64": np.ones((1, 64), dtype=f32),
        "onescol": np.ones((P, H), dtype=f32).astype(ndt),
    }
    return consts


def kernel(x, feature, Wc, bc, Wf, bf, Wp, bp, _trace=False, _out=None):
    x = np.asarray(x, dtype=np.float32)
    feature = np.asarray(feature, dtype=np.float32)
    if "nc" not in _CACHE:
        _CACHE["nc"] = _build()
    nc = _CACHE["nc"]
    consts = _prep_consts(
        np.asarray(Wc, np.float32), np.asarray(bc, np.float32),
        np.asarray(Wf, np.float32), np.asarray(bf, np.float32),
        np.asarray(Wp, np.float32), np.asarray(bp, np.float32),
    )
    B = x.shape[0]
    ndt = _np_dt()
    in_maps = [
        {"xb": np.ascontiguousarray(x[b]).astype(ndt),
         "fb": np.ascontiguousarray(feature[b]).astype(ndt), **consts}
        for b in range(B)
    ]
    res = run_bass_kernel_spmd(nc, in_maps, list(range(B)), trace=_trace)
    if _out is not None:
        _out.append(res)
    out = np.stack([res.results[b]["ob"] for b in range(B)], axis=0)
    return out
